# revision 18
# baseline (speedup 1.0000x reference)
"""Trainium2 Bass kernel for the Dihedral energy problem.

Contract: kernel(**inputs) takes the FULL unsharded inputs (numpy arrays,
same keys as reference.setup_inputs()) and returns the FULL [1000] f32
per-molecule energy vector.

Strategy (8 NeuronCores, SPMD):
  - Shard the dihedral dimension M=2,000,000 across 8 cores (data parallel),
    padding to 8*128*K_TOT so each partition owns a contiguous run.
  - Gather resolution: this container's SWDGE firmware does not support
    per-element indirect DMA (indirect_dma_start lowers to one strip
    descriptor per partition — verified empirically; 3-D dest APs crash the
    device, dma_gather requires int16 indices and 256-byte elements), so the
    random-access gathers (atom rows by mapping, parameter rows by type
    tuple) are resolved on the host during input staging and shipped as
    per-core contiguous streams.
  - Device per chunk: stream atom rows (x,y,z,type) for the 4 dihedral
    sites + 9-lane parameter rows (cos t_j, sin t_j, k_j), then DVE-only
    math: bond vectors, cross products, dots; cos/sin(theta) from
    x = n1.n2, y = |b2|(b1.n2) with a +1e-30 guard (reproduces
    atan2(0,0) = 0); multiple angles by polynomial identities (no ACT
    transcendentals except two domain-safe Sqrt); V; then a sorted-segment
    masked reduction into [P, R] per-molecule partials.
  - Degenerate dihedrals (atom1 == atom2 -> reference NaN via 0/0) are
    flagged with +1e30 on V; the host maps partials >= 1e29 to NaN.
  - Host: final scatter of the 8*128*R partials into the [1000] output
    (the cross-core all-reduce tail).
"""

import numpy as np
from contextlib import ExitStack

import concourse.bass as bass
import concourse.tile as tile
from concourse import bacc, mybir
from concourse.bass_utils import run_bass_kernel_spmd

F32 = mybir.dt.float32

P = 128
N_CORES = 8
N_ATOMS = 500_000
M = 2_000_000
NT = 25
NMOL = 1000

K_TOT = 1956                # free-dim elements per partition per core
PER_CORE = P * K_TOT        # 250368
M_PAD = N_CORES * PER_CORE  # 2002944
KC = 326                    # chunk width (free dim) -> 6 chunks
N_CHUNKS = K_TOT // KC
SENTINEL = 2000.0           # molecule id for padding elements (>= NMOL)
BIG = 1e30                  # collision flag added to V (host -> NaN)

_program_cache: dict = {}


def _build_program(R: int, reps: int = 1) -> bass.Bass:
    nc = bacc.Bacc("TRN2", target_bir_lowering=False, debug=False,
                   num_devices=N_CORES)

    adata = nc.declare_dram_parameter("adata", [4, PER_CORE, 4], F32,
                                      isOutput=False)
    pdata = nc.declare_dram_parameter("pdata", [PER_CORE, 9], F32,
                                      isOutput=False)
    batch = nc.declare_dram_parameter("batch", [PER_CORE], F32,
                                      isOutput=False)
    firsts = nc.declare_dram_parameter("firsts", [P, R], F32, isOutput=False)
    acc_out = nc.declare_dram_parameter("acc", [P, R], F32, isOutput=True)

    # 2-D APs with long contiguous per-partition runs (k and c collapse)
    adata_r = adata[:].rearrange("j (p k) c -> j p (k c)", p=P)  # [4,P,4*K_TOT]
    pdata_r = pdata[:].rearrange("(p k) c -> p (k c)", p=P)      # [P,9*K_TOT]
    batch_r = batch[:].rearrange("(p k) -> p k", p=P)            # [P,K_TOT]

    with tile.TileContext(nc) as tc, ExitStack() as ctx:
        io = ctx.enter_context(tc.tile_pool(name="io", bufs=2))
        tp = ctx.enter_context(tc.tile_pool(name="tp", bufs=1))
        ap = ctx.enter_context(tc.tile_pool(name="accp", bufs=1))

        firsts_t = ap.tile([P, R], F32)
        nc.sync.dma_start(firsts_t[:], firsts[:])
        # DVE-domain copy so mask compares don't add a second DMA-lane wait
        firsts_v = ap.tile([P, R], F32)
        nc.vector.tensor_copy(firsts_v[:], firsts_t[:])
        acc_t = ap.tile([P, R], F32)

        for rep in range(reps):
            nc.vector.memset(acc_t[:], 0.0)
            _chunk_loop(nc, R, adata_r, pdata_r, batch_r, io, tp,
                        firsts_v, acc_t)

        nc.sync.dma_start(acc_out[:], acc_t[:])

    nc.compile()
    return nc


def _chunk_loop(nc, R, adata_r, pdata_r, batch_r, io, tp, firsts_v, acc_t):
    MUL = mybir.AluOpType.mult
    ADD = mybir.AluOpType.add
    SUB = mybir.AluOpType.subtract
    EQ = mybir.AluOpType.is_equal

    for c in range(N_CHUNKS):
        sl = slice(c * KC, (c + 1) * KC)

        # ---- streamed loads (all contiguous-per-partition HWDGE) ----
        A = []
        for j in range(4):
            t = io.tile([P, 4 * KC], F32, tag=f"A{j}", name=f"A{j}")
            nc.sync.dma_start(t[:], adata_r[j, :, 4 * c * KC:4 * (c + 1) * KC])
            A.append(t)
        Pt = io.tile([P, 9 * KC], F32, tag="Pt")
        nc.sync.dma_start(Pt[:], pdata_r[:, 9 * c * KC:9 * (c + 1) * KC])
        bt = io.tile([P, KC], F32, tag="bt")
        nc.sync.dma_start(bt[:], batch_r[:, sl])

        def comp(tl, i):
            return tl[:].rearrange("p (k c) -> p k c", c=4)[:, :, i]

        def pcomp(i):
            return Pt[:].rearrange("p (k c) -> p k c", c=9)[:, :, i]

        # ---- bond vectors (lane 3 = type diff junk, unused) ----
        D1 = tp.tile([P, 4 * KC], F32, tag="D1")
        D2 = tp.tile([P, 4 * KC], F32, tag="D2")
        D3 = tp.tile([P, 4 * KC], F32, tag="D3")
        nc.vector.tensor_tensor(out=D1[:], in0=A[1][:], in1=A[0][:], op=SUB)
        nc.vector.tensor_tensor(out=D2[:], in0=A[2][:], in1=A[1][:], op=SUB)
        nc.vector.tensor_tensor(out=D3[:], in0=A[3][:], in1=A[2][:], op=SUB)

        def new(tag):
            return tp.tile([P, KC], F32, tag=tag, name=tag)

        def tt(o, a, b, op):
            nc.vector.tensor_tensor(
                out=o if isinstance(o, bass.AP) else o[:],
                in0=a if isinstance(a, bass.AP) else a[:],
                in1=b if isinstance(b, bass.AP) else b[:], op=op)

        def ts(o, a, s1, op0, s2=None, op1=None):
            kw = dict(scalar2=s2, op1=op1) if op1 is not None \
                else dict(scalar2=None)
            nc.vector.tensor_scalar(
                out=o if isinstance(o, bass.AP) else o[:],
                in0=a if isinstance(a, bass.AP) else a[:],
                scalar1=s1, op0=op0, **kw)

        def cross(pre, u, v):
            outs = []
            for (i, j, k) in ((0, 1, 2), (1, 2, 0), (2, 0, 1)):
                w1 = new(f"{pre}w1")
                w2 = new(f"{pre}w2")
                tt(w1, comp(u, j), comp(v, k), MUL)
                tt(w2, comp(u, k), comp(v, j), MUL)
                o = new(f"{pre}c{i}")
                tt(o, w1, w2, SUB)
                outs.append(o)
            return outs

        n1 = cross("n1", D1, D2)
        n2 = cross("n2", D2, D3)

        def dot3(pre, ua, ub):
            m0 = new(f"{pre}m0")
            m1 = new(f"{pre}m1")
            tt(m0, ua[0], ub[0], MUL)
            tt(m1, ua[1], ub[1], MUL)
            tt(m0, m0, m1, ADD)
            tt(m1, ua[2], ub[2], MUL)
            o = new(f"{pre}d")
            tt(o, m0, m1, ADD)
            return o

        x = dot3("x", n1, n2)
        d1c = [comp(D1, i) for i in range(3)]
        d2c = [comp(D2, i) for i in range(3)]
        s = dot3("s", d1c, n2)
        q = dot3("q", d2c, d2c)

        sq = new("sq")
        nc.scalar.activation(sq[:], q[:], mybir.ActivationFunctionType.Sqrt)
        y = new("y")
        tt(y, sq, s, MUL)

        x2 = new("x2")
        tt(x2, x, x, MUL)
        r2 = new("r2")
        tt(r2, y, y, MUL)
        tt(r2, r2, x2, ADD)
        r = new("r")
        nc.scalar.activation(r[:], r2[:], mybir.ActivationFunctionType.Sqrt)

        # collision flag: q == 0 <=> atom1 == atom2 (reference NaN case)
        zq = new("zq")
        ts(zq, q, 0.0, EQ)

        # cos/sin(theta) with +1e-30 guard: x=y=0 -> cos=1, sin=0 (atan2(0,0))
        num = new("num")
        ts(num, x, 1e-30, ADD)
        den = new("den")
        ts(den, r, 1e-30, ADD)
        invr = new("invr")
        nc.vector.reciprocal(invr[:], den[:])
        cs = new("cs")
        tt(cs, num, invr, MUL)
        sn = new("sn")
        tt(sn, y, invr, MUL)

        # multiple angles
        sn2 = new("sn2")
        tt(sn2, sn, sn, MUL)
        cos2 = new("cos2")
        ts(cos2, sn2, -2.0, MUL, 1.0, ADD)
        csn = new("csn")
        tt(csn, cs, sn, MUL)
        sin2 = new("sin2")
        ts(sin2, csn, 2.0, MUL)
        w2t = new("w2t")
        ts(w2t, sn2, -4.0, MUL, 1.0, ADD)
        cos3 = new("cos3")
        tt(cos3, cs, w2t, MUL)
        w3t = new("w3t")
        ts(w3t, sn2, -4.0, MUL, 3.0, ADD)
        sin3 = new("sin3")
        tt(sin3, sn, w3t, MUL)

        # ---- V = sum_j k_j * (1 - (cosj*ct_j + sinj*st_j)) + BIG*zq ----
        cos_l = [cs, cos2, cos3]
        sin_l = [sn, sin2, sin3]
        msum = new("msum")
        w4 = new("w4")
        cj = new("cj")
        for j in range(3):
            tt(cj, cos_l[j], pcomp(3 * j), MUL)
            tt(w4, sin_l[j], pcomp(3 * j + 1), MUL)
            tt(cj, cj, w4, ADD)
            if j == 0:
                tt(msum, cj, pcomp(2), MUL)
            else:
                tt(w4, cj, pcomp(3 * j + 2), MUL)
                tt(msum, msum, w4, ADD)
        ksum = new("ksum")
        tt(ksum, pcomp(2), pcomp(5), ADD)
        tt(ksum, ksum, pcomp(8), ADD)
        V = new("V")
        tt(V, ksum, msum, SUB)
        ts(w4, zq, BIG, MUL)
        tt(V, V, w4, ADD)

        # ---- sorted-segment masked reduction ----
        mask = new("mask")
        scr = new("scr")
        red = tp.tile([P, R], F32, tag="red")
        for rr in range(R):
            tt(mask, bt, firsts_v[:, rr:rr + 1].to_broadcast([P, KC]), EQ)
            tt(scr, mask, V, MUL)
            nc.vector.tensor_reduce(out=red[:, rr:rr + 1], in_=scr[:],
                                    axis=mybir.AxisListType.X, op=ADD)
        tt(acc_t, acc_t, red, ADD)


def _get_program(R: int) -> bass.Bass:
    if R not in _program_cache:
        _program_cache[R] = _build_program(R)
    return _program_cache[R]


def _prep(pos, theta_0, k_0, theta_1, k_1, theta_2, k_2,
          mapping, atom_types, mapping_batch):
    """Host-side staging: resolve the random-access gathers into per-core
    contiguous streams (the device firmware has no per-element gather)."""
    atom_data = np.empty((N_ATOMS, 4), np.float32)
    atom_data[:, :3] = np.asarray(pos, np.float32)
    atom_data[:, 3] = np.asarray(atom_types, np.float32)

    params = np.empty((NT ** 4, 9), np.float32)
    for j, (th, kk) in enumerate(((theta_0, k_0), (theta_1, k_1),
                                  (theta_2, k_2))):
        tf = np.asarray(th, np.float32).reshape(-1).astype(np.float64)
        params[:, 3 * j] = np.cos(tf).astype(np.float32)
        params[:, 3 * j + 1] = np.sin(tf).astype(np.float32)
        params[:, 3 * j + 2] = np.asarray(kk, np.float32).reshape(-1)

    maps_pad = np.empty((4, M_PAD), np.int64)
    maps_pad[:, :M] = np.asarray(mapping, np.int64)
    maps_pad[:, M:] = np.array([[0], [1], [2], [3]])

    adata = atom_data[maps_pad]                   # [4, M_PAD, 4]

    at = np.asarray(atom_types, np.int64)
    t0, t1, t2, t3 = (at[maps_pad[i]] for i in range(4))
    flat = ((t0 * NT + t1) * NT + t2) * NT + t3
    pdata = params[flat]                          # [M_PAD, 9]

    batch_pad = np.full(M_PAD, SENTINEL, np.float32)
    batch_pad[:M] = np.asarray(mapping_batch, np.float32)
    return adata, pdata, batch_pad


def _max_span(mapping_batch) -> int:
    mb = np.asarray(mapping_batch)
    spans = []
    for start in range(0, M_PAD, K_TOT):
        if start >= M:
            break
        end = min(start + K_TOT, M)
        spans.append(int(mb[end - 1] - mb[start] + 1))
    return max(spans)


def kernel(pos, theta_0, k_0, theta_1, k_1, theta_2, k_2,
           mapping, atom_types, mapping_batch, _trace=False):
    adata, pdata, batch_pad = _prep(
        pos, theta_0, k_0, theta_1, k_1, theta_2, k_2,
        mapping, atom_types, mapping_batch)

    R = max(4, _max_span(mapping_batch))
    nc = _get_program(R)

    in_maps = []
    firsts_all = []
    for c in range(N_CORES):
        lo, hi = c * PER_CORE, (c + 1) * PER_CORE
        b = batch_pad[lo:hi]
        first = b[::K_TOT].astype(np.float32)
        firsts = (first[:, None]
                  + np.arange(R, dtype=np.float32)[None, :]).astype(np.float32)
        firsts_all.append(firsts)
        in_maps.append({
            "adata": np.ascontiguousarray(adata[:, lo:hi, :]),
            "pdata": np.ascontiguousarray(pdata[lo:hi]),
            "batch": b,
            "firsts": firsts,
        })

    res = run_bass_kernel_spmd(nc, in_maps, core_ids=list(range(N_CORES)),
                               trace=_trace)

    out = np.zeros(NMOL, np.float32)
    for c in range(N_CORES):
        acc = res.results[c]["acc"]
        ids = firsts_all[c].astype(np.int64).reshape(-1)
        vals = acc.reshape(-1)
        ok = (ids >= 0) & (ids < NMOL)
        np.add.at(out, ids[ok], vals[ok])
    out[np.abs(out) >= 1e29] = np.nan
    if _trace:
        kernel._last_results = res
    return out
